# revision 1
# baseline (speedup 1.0000x reference)
"""Low-rank sparse attention, sharded over 8 NeuronCores.

Sharding: data-parallel over batch B (=2) and query-block-parallel over L
(4 blocks of 512 per batch) -> 8 shards, one per core. k/v for a batch and
the small low-rank factors are replicated on the cores that need them.
Each core computes its 512 query rows end-to-end (projections, scores,
top-64 softmax, attention, output projection) with no cross-core reduction;
the host only slices inputs and concatenates outputs.
"""

import numpy as np

# Hardcoded problem shapes (self-contained; do not read spec/reference).
B, L, S, D = 2, 2048, 2048, 1024
H, DH, RANK, TOPK = 16, 64, 128, 64
SCALE = DH ** -0.5
NCORES = 8
LBLK = L // 4  # 512 query rows per core


def _shard_plan():
    # core i -> (batch, l-start)
    return [(i // 4, (i % 4) * LBLK) for i in range(NCORES)]


def _device_fn(jnp, lax):
    def f(q, k, v, Uq, Vq, bq, Uk, Vk, bk, Uv, Vv, bv, Uo, Vo, bo):
        # q: [LBLK, D], k/v: [S, D]
        def proj(x, U, V, b):
            return (x @ U) @ V.T + b

        def heads(x, T):
            return x.reshape(T, H, DH).transpose(1, 0, 2)  # [H, T, DH]

        qh = heads(proj(q, Uq, Vq, bq), LBLK)   # [H, LBLK, DH]
        kh = heads(proj(k, Uk, Vk, bk), S)      # [H, S, DH]
        vh = heads(proj(v, Uv, Vv, bv), S)      # [H, S, DH]

        scores = jnp.einsum("hld,hsd->hls", qh, kh) * jnp.float32(SCALE)
        flat = scores.reshape(-1, S)            # [H*LBLK, S]

        # top-64 softmax == full softmax with entries below the 64th-largest
        # value masked out (no scatter needed; exact same selection as
        # masking non-topk to -inf since random scores have no ties).
        vals = lax.top_k(flat, TOPK)[0]         # [N, 64] descending
        thresh = vals[:, -1:]                   # 64th largest per row
        mask = flat >= thresh
        mx = vals[:, :1]
        e = jnp.where(mask, jnp.exp(flat - mx), 0.0)
        p = e / e.sum(axis=-1, keepdims=True)

        out_h = jnp.einsum("hls,hsd->hld", p.reshape(H, LBLK, S), vh)
        out = out_h.transpose(1, 0, 2).reshape(LBLK, D)
        return proj(out, Uo, Vo, bo)

    return f


def kernel(**inputs: np.ndarray) -> np.ndarray:
    import jax
    import jax.numpy as jnp
    from jax import lax

    q = np.asarray(inputs["q"], np.float32)
    k = np.asarray(inputs["k"], np.float32)
    v = np.asarray(inputs["v"], np.float32)
    factors = {n: np.asarray(inputs[n], np.float32)
               for n in ("Uq", "Vq", "bq", "Uk", "Vk", "bk",
                         "Uv", "Vv", "bv", "Uo", "Vo", "bo")}

    plan = _shard_plan()
    # Stack per-core shards on a leading device axis.
    q_sh = np.stack([q[b, l0:l0 + LBLK] for b, l0 in plan])      # [8, 512, D]
    k_sh = np.stack([k[b] for b, _ in plan])                     # [8, S, D]
    v_sh = np.stack([v[b] for b, _ in plan])                     # [8, S, D]

    f = _device_fn(jnp, lax)
    fp = jax.pmap(
        f,
        in_axes=(0, 0, 0) + (None,) * 12,
        devices=jax.devices()[:NCORES],
    )
    res = fp(q_sh, k_sh, v_sh,
             factors["Uq"], factors["Vq"], factors["bq"],
             factors["Uk"], factors["Vk"], factors["bk"],
             factors["Uv"], factors["Vv"], factors["bv"],
             factors["Uo"], factors["Vo"], factors["bo"])
    res = np.asarray(res)                                        # [8, 512, D]

    out = np.empty((B, L, D), np.float32)
    for i, (b, l0) in enumerate(plan):
        out[b, l0:l0 + LBLK] = res[i]
    return out


if __name__ == "__main__":
    rng = np.random.default_rng(0)
    dummy = {
        "q": rng.standard_normal((B, L, D), dtype=np.float32),
        "k": rng.standard_normal((B, S, D), dtype=np.float32),
        "v": rng.standard_normal((B, S, D), dtype=np.float32),
    }
    for n in "qkvo":
        dummy[f"U{n}"] = rng.standard_normal((D, RANK), dtype=np.float32) * 0.05
        dummy[f"V{n}"] = rng.standard_normal((D, RANK), dtype=np.float32) * 0.05
        dummy[f"b{n}"] = np.zeros((D,), np.float32)
    o = kernel(**dummy)
    print("ok", o.shape, float(np.abs(o).max()))
